# revision 1
# baseline (speedup 1.0000x reference)
"""GQA attention block (RMSNorm + QKV proj + partial RoPE + causal GQA
attention + XSA correction + out proj) on 8 trn2 NeuronCores.

Sharding: 2 batches x 4 KV-groups (each core: 1 batch, 1 kv head, 4 q heads).
Each core computes a partial output (its 4 heads through its wo column slice);
the host sums the 4 partials per batch.

Layout strategy: everything transposed ([feature, token]) so the contraction
dim of every matmul lands on partitions.  fp32r matmuls for QKV/scores
(operand tiles are native float32r, produced by engine ops so walrus sees
them as rounded), bf16 for P@V and the output projection.  Softmax without
max-subtraction (logits are ~N(0,1)); causal masking zeroes invalid P
entries with gpsimd.affine_select after exp.
"""

import sys

for _p in ("/opt/trn_rl_repo", "/root/.axon_site/_ro/trn_rl_repo"):
    if _p not in sys.path:
        sys.path.append(_p)

import numpy as np
import ml_dtypes

import concourse.bass as bass
import concourse.bacc as bacc
import concourse.mybir as mybir
import concourse.tile as tile
from concourse.bass_utils import run_bass_kernel_spmd
from concourse.masks import make_identity

F32 = mybir.dt.float32
F32R = mybir.dt.float32r
BF16 = mybir.dt.bfloat16

B, T, D = 2, 2048, 2048
NH, NKV, HD = 16, 4, 128
RD = 64  # rope dims
NH_L = NH // NKV           # 4 q heads per core
EL = (NH_L + 2) * HD       # 768: q0..q3, k, v
TC = 512                   # token chunk
NTC = T // TC              # 4
DC = D // 128              # 16 contraction chunks
S128 = float(1.0 / np.sqrt(HD))
EPS = 1e-6

_CACHE = {}


def _build_nc():
    nc = bacc.Bacc("TRN2", target_bir_lowering=False, debug=False)

    xT = nc.declare_dram_parameter("xT", [D, T], F32, isOutput=False)
    wT = nc.declare_dram_parameter("wqkvT", [D, EL], F32, isOutput=False)
    woT = nc.declare_dram_parameter("woT", [NH_L * HD, D], BF16, isOutput=False)
    csP = nc.declare_dram_parameter("cs", [128, T], F32, isOutput=False)
    outp = nc.declare_dram_parameter("out", [T, D], F32, isOutput=True)

    ACT = mybir.ActivationFunctionType

    with tile.TileContext(nc) as tc:
        with (
            nc.allow_low_precision(reason="fp32r feeds matmul; tolerances ok"),
            tc.tile_pool(name="singles", bufs=1) as sg,
            tc.tile_pool(name="stream", bufs=2) as st,
            tc.tile_pool(name="ps", bufs=1, space="PSUM") as ps,
        ):
            # ---- persistent tiles ----
            w_sb = sg.tile([128, DC * EL], F32R, tag="w")
            cos_sb = sg.tile([RD, T], F32, tag="cos")
            sin_sb = sg.tile([RD, T], F32, tag="sin")
            ident = sg.tile([128, 128], F32, tag="ident")
            ones_f = sg.tile([128, 1], F32, tag="ones_f")
            ones_rf = sg.tile([1, 128], F32, tag="ones_rf")
            ones_c = sg.tile([128, 1], F32R, tag="ones_c")
            ones_r = sg.tile([1, 128], F32R, tag="ones_r")
            ones_cb = sg.tile([128, 1], BF16, tag="ones_cb")
            eps_t = sg.tile([1, 1], F32, tag="eps_t")
            qhat = [
                [sg.tile([128, TC], F32R, tag=f"qh{h}_{j}", name=f"qh{h}_{j}")
                 for j in range(NTC)]
                for h in range(NH_L)
            ]
            khat = [sg.tile([128, TC], F32R, tag=f"kh{j}", name=f"kh{j}")
                    for j in range(NTC)]
            vhat = [sg.tile([128, TC], F32, tag=f"vh{j}", name=f"vh{j}")
                    for j in range(NTC)]
            vtok = [sg.tile([128, TC], BF16, tag=f"vt{j}", name=f"vt{j}")
                    for j in range(NTC)]
            aout = [
                [sg.tile([128, TC], BF16, tag=f"ao{h}_{j}", name=f"ao{h}_{j}")
                 for j in range(NTC)]
                for h in range(NH_L)
            ]
            rvns = [sg.tile([1, TC], F32, tag=f"rvns{j}", name=f"rvns{j}")
                    for j in range(NTC)]

            nc.sync.dma_start(out=cos_sb, in_=csP[0:RD, :])
            nc.sync.dma_start(out=sin_sb, in_=csP[RD:128, :])
            make_identity(nc, ident)
            nc.vector.memset(ones_f, 1.0)
            nc.vector.memset(ones_cb, 1.0)
            nc.vector.memset(eps_t, EPS)
            nc.vector.memset(ones_rf, 1.0)
            nc.scalar.copy(ones_c, ones_f)           # rounded fp32r ones
            nc.scalar.copy(ones_r, ones_rf)
            for i in range(DC):
                wld = st.tile([128, EL], F32, tag="ld", bufs=2, name=f"wld{i}")
                nc.sync.dma_start(
                    out=wld, in_=wT[i * 128:(i + 1) * 128, :]
                )
                nc.gpsimd.tensor_copy(w_sb[:, i * EL:(i + 1) * EL], wld)

            # ========== QKV + rmsnorm scale + rope + attention, per chunk ======
            for j in range(NTC):
                js = slice(j * TC, (j + 1) * TC)
                ps_qkv = [ps.tile([128, TC], F32, tag="A", bufs=6,
                                  name=f"psqkv{j}_{_e}") for _e in range(6)]
                ps_rs = ps.tile([1, TC], F32, tag="S", bufs=2)
                for i in range(DC):
                    xt = st.tile([128, EL], F32, tag="ld", bufs=2, name=f"xt{j}_{i}")
                    nc.sync.dma_start(
                        out=xt[:, 0:TC], in_=xT[i * 128:(i + 1) * 128, js]
                    )
                    xtr = st.tile([128, TC], F32R, tag="xtr", bufs=2)
                    nc.gpsimd.tensor_copy(xtr, xt[:, 0:TC])
                    for e in range(6):
                        nc.tensor.matmul(
                            ps_qkv[e],
                            w_sb[:, i * EL + e * 128: i * EL + (e + 1) * 128],
                            xtr,
                            start=(i == 0), stop=(i == DC - 1),
                        )
                    x2r = st.tile([128, TC], F32R, tag="x2r", bufs=1)
                    nc.vector.tensor_mul(x2r, xt[:, 0:TC], xt[:, 0:TC])
                    nc.tensor.matmul(
                        ps_rs, ones_c, x2r,
                        start=(i == 0), stop=(i == DC - 1),
                    )
                # evac raw projections (rounds into fp32r tiles)
                for h in range(NH_L):
                    nc.scalar.copy(qhat[h][j], ps_qkv[h])
                nc.scalar.copy(khat[j], ps_qkv[4])
                nc.scalar.copy(vhat[j], ps_qkv[5])

                # rs = 1/sqrt(mean(x^2)+eps), broadcast across partitions
                sq = st.tile([1, TC], F32, tag="sq", bufs=1)
                nc.scalar.activation(sq, ps_rs, ACT.Sqrt, scale=1.0 / D, bias=eps_t)
                rs_t = st.tile([1, TC], F32, tag="rs_t", bufs=1)
                nc.vector.reciprocal(rs_t, sq)
                rsb = st.tile([128, TC], F32, tag="rsb", bufs=2)
                nc.gpsimd.partition_broadcast(rsb, rs_t)

                # fold rs into rope tables (in place, this chunk's columns)
                nc.vector.tensor_mul(cos_sb[:, js], cos_sb[:, js], rsb[0:RD])
                nc.vector.tensor_mul(sin_sb[:, js], sin_sb[:, js], rsb[0:RD])

                # rope + rs scaling for q heads and k (swap halves via dma)
                for th in [qhat[h][j] for h in range(NH_L)] + [khat[j]]:
                    t2s = st.tile([RD, TC], F32R, tag="t2s", bufs=2)
                    t1 = st.tile([RD, TC], F32, tag="t1", bufs=1)
                    nc.sync.dma_start(out=t2s[0:32], in_=th[32:64])
                    nc.sync.dma_start(out=t2s[32:64], in_=th[0:32])
                    nc.gpsimd.tensor_mul(t2s, t2s, sin_sb[:, js])
                    nc.vector.tensor_mul(t1, th[0:RD], cos_sb[:, js])
                    nc.vector.tensor_add(th[0:RD], t1, t2s)
                    nc.vector.tensor_mul(th[RD:128], th[RD:128], rsb[RD:128])

                # v-hat = v * rs ; 1/(sum(v^2)+eps) ; token-major v (bf16)
                nc.vector.tensor_mul(vhat[j], vhat[j], rsb)
                vsq = st.tile([128, TC], F32R, tag="vsq", bufs=1)
                nc.gpsimd.tensor_mul(vsq, vhat[j], vhat[j])
                ps_vns = ps.tile([1, TC], F32, tag="S", bufs=2)
                nc.tensor.matmul(ps_vns, ones_c, vsq, start=True, stop=True)
                vnse = st.tile([1, TC], F32, tag="vnse", bufs=1)
                nc.scalar.activation(vnse, ps_vns, ACT.Identity, bias=eps_t, scale=1.0)
                nc.vector.reciprocal(rvns[j], vnse)

                ps_vt = ps.tile([128, TC], F32, tag="A", bufs=6)
                for kk in range(TC // 128):
                    nc.tensor.transpose(
                        ps_vt[:, kk * 128:(kk + 1) * 128],
                        vhat[j][:, kk * 128:(kk + 1) * 128],
                        ident,
                    )
                nc.scalar.copy(vtok[j], ps_vt)

                # ---------------- attention for this q chunk -------------------
                for h in range(NH_L):
                    nkt = 4 * (j + 1)
                    ps_pv = ps.tile([128, TC], F32, tag="A", bufs=6)
                    ps_sum = ps.tile([1, TC], F32, tag="S", bufs=2)
                    for kt in range(nkt):
                        jk = kt // 4
                        ps_sc = ps.tile([128, TC], F32, tag="A", bufs=6)
                        nc.tensor.matmul(
                            ps_sc,
                            khat[jk][:, (kt % 4) * 128:(kt % 4 + 1) * 128],
                            qhat[h][j],
                            start=True, stop=True,
                        )
                        pT = st.tile([128, TC], BF16, tag="pT", bufs=3)
                        nc.scalar.activation(pT, ps_sc, ACT.Exp, scale=S128)
                        if kt >= 4 * j:  # diagonal-block tiles: causal zeroing
                            m = kt - 4 * j
                            nc.gpsimd.affine_select(
                                out=pT, in_=pT,
                                compare_op=mybir.AluOpType.is_ge,
                                fill=0.0,
                                base=-m * 128,
                                pattern=[[1, TC]],
                                channel_multiplier=-1,
                            )
                        nc.tensor.matmul(
                            ps_sum, ones_cb, pT,
                            start=(kt == 0), stop=(kt == nkt - 1),
                        )
                        nc.tensor.matmul(
                            ps_pv,
                            vtok[jk][:, (kt % 4) * 128:(kt % 4 + 1) * 128],
                            pT,
                            start=(kt == 0), stop=(kt == nkt - 1),
                        )
                    # normalization + XSA correction
                    inv = st.tile([1, TC], F32R, tag="inv", bufs=2)
                    nc.vector.reciprocal(inv, ps_sum)
                    pvsb = st.tile([128, TC], F32, tag="pvsb", bufs=2)
                    nc.scalar.copy(pvsb, ps_pv)
                    tu = st.tile([128, TC], F32R, tag="tu", bufs=1)
                    nc.gpsimd.tensor_mul(tu, pvsb, vhat[j])
                    ps_dot = ps.tile([1, TC], F32, tag="S", bufs=2)
                    nc.tensor.matmul(ps_dot, ones_c, tu, start=True, stop=True)
                    fu = st.tile([1, TC], F32, tag="fu", bufs=1)
                    nc.vector.tensor_mul(fu, ps_dot, rvns[j])
                    fui = st.tile([1, TC], F32R, tag="fui", bufs=1)
                    nc.vector.tensor_mul(fui, fu, inv)
                    ps_fb = ps.tile([128, TC], F32, tag="A", bufs=6)
                    nc.tensor.matmul(ps_fb, ones_r, fui, start=True, stop=True)
                    ps_ib = ps.tile([128, TC], F32, tag="A", bufs=6)
                    nc.tensor.matmul(ps_ib, ones_r, inv, start=True, stop=True)
                    m1 = st.tile([128, TC], F32, tag="mm", bufs=3, name="m1")
                    nc.vector.tensor_mul(m1, pvsb, ps_ib)
                    m2 = st.tile([128, TC], F32, tag="mm", bufs=3, name="m2")
                    nc.vector.tensor_mul(m2, vhat[j], ps_fb)
                    nc.vector.tensor_sub(aout[h][j], m1, m2)

            # ================= output projection ===========================
            for m in range(4):
                ms = slice(m * TC, (m + 1) * TC)
                wom = [st.tile([128, TC], BF16, tag="wo", bufs=4,
                               name=f"wom{m}_{_h}") for _h in range(NH_L)]
                for h in range(NH_L):
                    nc.sync.dma_start(out=wom[h], in_=woT[h * 128:(h + 1) * 128, ms])
                for tt in range(T // 128):
                    ps_o = ps.tile([128, TC], F32, tag="A", bufs=6)
                    for h in range(NH_L):
                        nc.tensor.matmul(
                            ps_o,
                            aout[h][tt // 4][:, (tt % 4) * 128:(tt % 4 + 1) * 128],
                            wom[h],
                            start=(h == 0), stop=(h == NH_L - 1),
                        )
                    osb = st.tile([128, TC], F32, tag="osb", bufs=4)
                    if tt % 2 == 0:
                        nc.scalar.copy(osb, ps_o)
                    else:
                        nc.vector.tensor_copy(osb, ps_o)
                    nc.sync.dma_start(out=outp[tt * 128:(tt + 1) * 128, ms], in_=osb)

    nc.compile()
    return nc


def _host_inputs(x, cos, sin, w_norm, wq, wk, wv, wo):
    """Build the 8 per-core input maps (host-side layout prep only)."""
    wn = w_norm.astype(np.float32)
    cosT = cos.T.astype(np.float32)                                # [64, T]
    sinT = sin.T.astype(np.float32)
    sinS = np.concatenate([-sinT[:32], sinT[32:]], axis=0)         # [64, T]
    cs = np.ascontiguousarray(np.concatenate([cosT, sinS], axis=0))  # [128, T]
    xTs = [np.ascontiguousarray(x[b].T.astype(np.float32)) for b in range(B)]
    in_maps = []
    for c in range(8):
        b, g = divmod(c, 4)
        wq_s = wq[g * NH_L * HD:(g + 1) * NH_L * HD] * wn[None, :]
        wk_s = wk[g * HD:(g + 1) * HD] * wn[None, :]
        wv_s = wv[g * HD:(g + 1) * HD] * wn[None, :]
        wqkvT = np.ascontiguousarray(
            np.concatenate([wq_s, wk_s, wv_s], axis=0).T.astype(np.float32)
        )                                                          # [D, 768]
        woT_s = np.ascontiguousarray(
            wo[:, g * NH_L * HD:(g + 1) * NH_L * HD].T
        ).astype(ml_dtypes.bfloat16)                               # [512, D]
        in_maps.append({
            "xT": xTs[b],
            "wqkvT": wqkvT,
            "woT": woT_s,
            "cs": cs,
        })
    return in_maps


def kernel(x, cos, sin, w_norm, wq, wk, wv, wo, rope_dims=64, use_xsa=1,
           **_unused):
    if "nc" not in _CACHE:
        _CACHE["nc"] = _build_nc()
    nc = _CACHE["nc"]
    in_maps = _host_inputs(
        np.asarray(x), np.asarray(cos), np.asarray(sin), np.asarray(w_norm),
        np.asarray(wq), np.asarray(wk), np.asarray(wv), np.asarray(wo),
    )
    res_obj = run_bass_kernel_spmd(nc, in_maps, list(range(8)))
    _CACHE["last"] = res_obj
    res = res_obj.results
    out = np.zeros((B, T, D), dtype=np.float32)
    for c in range(8):
        b = c // 4
        out[b] += np.asarray(res[c]["out"], dtype=np.float32)
    return out



# revision 15
# speedup vs baseline: 1.2131x; 1.2131x over previous
"""GQA attention block (RMSNorm + QKV proj + partial RoPE + causal GQA
attention + XSA correction + out proj) on 8 trn2 NeuronCores.

Sharding: 2 batches x 4 KV-groups (each core: 1 batch, 1 kv head, 4 q heads).
Each core computes a partial output (its 4 heads through its wo column slice);
the host sums the 4 partials per batch.

v2 design (vs the fp32r baseline):
- all matmul operands bf16 (FWL weight loads, fast LDWEIGHTS, no gpsimd casts)
- causal mask folded into the score matmul as an identity @ mask-constant
  accumulation (exp of -1e6 underflows to 0) -- no gpsimd on the exp->PV path
- softmax denominator accumulated on the Vector engine (sumP += pT), one
  [1,TC] ones-matmul per (head,chunk) instead of one per key-tile
- rms scale rs computed from a row-major copy of x via tensor_tensor_reduce
  (no PE cycles, no x^2 elementwise muls)
- XSA + normalization via gpsimd partition_broadcast + DVE fast reciprocal
- software-pipelined issue order: next chunk's QKV projection and previous
  chunk's output projection matmuls are interleaved as "filler" into the
  scalar-bound attention phase so the PE never idles (keeps HAM at K=8/8)
"""

import sys
from collections import deque

for _p in ("/opt/trn_rl_repo", "/root/.axon_site/_ro/trn_rl_repo"):
    if _p not in sys.path:
        sys.path.append(_p)

import numpy as np
import ml_dtypes

import concourse.bass as bass
import concourse.bacc as bacc
import concourse.mybir as mybir
import concourse.tile as tile
from concourse.bass_utils import run_bass_kernel_spmd
from concourse.masks import make_identity

F32 = mybir.dt.float32
F32R = mybir.dt.float32r
BF16 = mybir.dt.bfloat16

B, T, D = 2, 2048, 2048
NH, NKV, HD = 16, 4, 128
RD = 64                    # rope dims
NH_L = NH // NKV           # 4 q heads per core
EL = (NH_L + 2) * HD       # 768: q0..q3, k, v
TC = 512                   # token chunk
NTC = T // TC              # 4
DC = D // 128              # 16 contraction chunks
S128 = float(1.0 / np.sqrt(HD))
EPS = 1e-6
MASKV = -1.0e6

_CACHE = {}


def _build_nc():
    nc = bacc.Bacc("TRN2", target_bir_lowering=False, debug=False)

    xT = nc.declare_dram_parameter("xT", [D, T], BF16, isOutput=False)
    xR = nc.declare_dram_parameter("xR", [T, D], BF16, isOutput=False)
    wT = nc.declare_dram_parameter("wqkvT", [D, EL], BF16, isOutput=False)
    woL = nc.declare_dram_parameter("woL", [128, NH_L * D], BF16, isOutput=False)
    csP = nc.declare_dram_parameter("cs", [128, T], F32, isOutput=False)
    outp = nc.declare_dram_parameter("out", [T, D], F32, isOutput=True)

    ACT = mybir.ActivationFunctionType
    ALU = mybir.AluOpType

    with tile.TileContext(nc) as tc:
        with (
            nc.allow_low_precision(reason="bf16 matmuls; tolerance 2e-2"),
            tc.tile_pool(name="singles", bufs=1) as sg,
            tc.tile_pool(name="stream", bufs=2) as st,
            tc.tile_pool(name="ps", bufs=1, space="PSUM") as ps,
        ):
            # ---------------- persistent tiles ----------------
            w_sb = sg.tile([128, DC * EL], BF16, tag="w")
            wo_sb = sg.tile([128, NH_L * D], BF16, tag="wo")
            cos_sb = sg.tile([RD, T], F32, tag="cos")
            sinS_sb = sg.tile([RD, T], F32, tag="sin")
            identf = sg.tile([128, 128], F32, tag="identf")
            identb = sg.tile([128, 128], BF16, tag="identb")
            ones_c = sg.tile([128, 1], F32R, tag="ones_c")
            ones_f = sg.tile([128, 1], F32, tag="ones_f")
            eps_t = sg.tile([128, 1], F32, tag="eps_t")
            maskM = [sg.tile([128, TC], BF16, tag=f"mask{m}", name=f"mask{m}")
                     for m in range(4)]
            qhat = [[sg.tile([128, TC], BF16, tag=f"qh{h}_{r}", name=f"qh{h}_{r}")
                     for r in range(2)] for h in range(NH_L)]
            khat = [sg.tile([128, TC], BF16, tag=f"kh{j}", name=f"kh{j}")
                    for j in range(NTC)]
            vhat = [sg.tile([128, TC], BF16, tag=f"vh{j}", name=f"vh{j}")
                    for j in range(NTC)]
            vtok = [sg.tile([128, TC], BF16, tag=f"vt{j}", name=f"vt{j}")
                    for j in range(NTC)]
            rvnsb = [sg.tile([128, TC], F32, tag=f"rvns{j}", name=f"rvns{j}")
                     for j in range(NTC)]
            sumP = [sg.tile([128, TC], F32R, tag=f"sp{h}", name=f"sp{h}")
                    for h in range(NH_L)]
            aout = [[sg.tile([128, TC], BF16, tag=f"ao{h}_{r}", name=f"ao{h}_{r}")
                     for r in range(2)] for h in range(NH_L)]
            rsb = [sg.tile([128, TC], F32, tag=f"rsb{r}", name=f"rsb{r}")
                   for r in range(2)]

            # ---------------- init ----------------
            nc.sync.dma_start(out=cos_sb, in_=csP[0:RD, :])
            nc.sync.dma_start(out=sinS_sb, in_=csP[RD:128, :])
            for h in range(NH_L):
                nc.sync.dma_start(out=wo_sb[:, h * D:(h + 1) * D],
                                  in_=woL[:, h * D:(h + 1) * D])
            for i in range(DC):
                nc.sync.dma_start(out=w_sb[:, i * EL:(i + 1) * EL],
                                  in_=wT[i * 128:(i + 1) * 128, :])
            make_identity(nc, identf)
            nc.gpsimd.tensor_copy(identb, identf)
            nc.vector.memset(ones_f, 1.0)
            nc.scalar.copy(ones_c, ones_f)
            nc.vector.memset(eps_t, EPS)
            for m in range(4):
                nc.vector.memset(maskM[m], 0.0)
                # mask[r, c] = 0 where c >= r + 128*m else MASKV
                nc.gpsimd.affine_select(
                    out=maskM[m], in_=maskM[m],
                    compare_op=ALU.is_ge, fill=MASKV,
                    base=-m * 128, pattern=[[1, TC]], channel_multiplier=-1,
                )

            # ---------------- filler machinery ----------------
            fill_q = deque()

            def emit_fill(n):
                done = 0
                while fill_q and done < n:
                    try:
                        next(fill_q[0])
                        done += 1
                    except StopIteration:
                        fill_q.popleft()

            def drain_fill():
                while fill_q:
                    try:
                        next(fill_q[0])
                    except StopIteration:
                        fill_q.popleft()

            # ---------------- chunk prep (QKV + rs + rope + vtok + vns) ----
            def prep_gen(jn):
                js = slice(jn * TC, (jn + 1) * TC)
                r = jn % 2

                # x row-major tiles -> per-token sum of squares -> rs row
                srow = ps.tile([1, TC], F32, tag="S", bufs=1, name=f"srow{jn}")
                for kk in range(4):
                    xr = st.tile([128, D], BF16, tag="xr", bufs=4,
                                 name=f"xr{jn}_{kk}")
                    nc.sync.dma_start(
                        out=xr, in_=xR[jn * TC + kk * 128: jn * TC + (kk + 1) * 128, :])
                    xsq = st.tile([128, D], BF16, tag="xsq", bufs=1)
                    ssq = st.tile([128, 1], F32, tag="ssq", bufs=8)
                    nc.scalar.activation(xsq, xr, ACT.Square, accum_out=ssq)
                    nc.tensor.matmul(
                        srow[0:1, kk * 128:(kk + 1) * 128], ssq, identf,
                        start=True, stop=True)
                    yield
                srow_sb = st.tile([1, TC], F32, tag="row", bufs=4, name="srow_sb")
                nc.scalar.copy(srow_sb, srow)
                ms_b = st.tile([128, TC], F32, tag="bc", bufs=8, name="ms_b")
                nc.gpsimd.partition_broadcast(ms_b, srow_sb)
                sq_b = st.tile([128, TC], F32, tag="bc", bufs=8, name="sq_b")
                nc.scalar.activation(sq_b, ms_b, ACT.Sqrt, scale=1.0 / D,
                                     bias=eps_t)
                nc.vector.reciprocal_approx_fast(rsb[r], sq_b)
                yield

                # xT tiles for the QKV contraction
                xts = []
                for i in range(DC):
                    xt = st.tile([128, TC], BF16, tag="xt", bufs=20,
                                 name=f"xt{jn}_{i}")
                    nc.sync.dma_start(out=xt, in_=xT[i * 128:(i + 1) * 128, js])
                    xts.append(xt)
                yield

                # QKV projection, output-major (one PSUM bank)
                for e in range(6):
                    qk = ps.tile([128, TC], F32, tag="QK", bufs=1,
                                 name=f"qk{jn}_{e}")
                    for i in range(DC):
                        nc.tensor.matmul(
                            qk,
                            w_sb[:, i * EL + e * 128: i * EL + (e + 1) * 128],
                            xts[i],
                            start=(i == 0), stop=(i == DC - 1),
                        )
                        if i % 2 == 1:
                            yield
                    if e < NH_L:
                        dest = qhat[e][r]
                    elif e == NH_L:
                        dest = khat[jn]
                    else:
                        dest = vhat[jn]
                    nc.vector.tensor_mul(dest, qk, rsb[r])
                    yield
                    # rope for q heads and k (not v)
                    if e <= NH_L:
                        t2 = st.tile([RD, TC], BF16, tag="t2", bufs=2)
                        nc.sync.dma_start(out=t2[0:32], in_=dest[32:64])
                        nc.sync.dma_start(out=t2[32:64], in_=dest[0:32])
                        nc.gpsimd.tensor_mul(t2, t2, sinS_sb[:, js])
                        t1 = st.tile([RD, TC], BF16, tag="t1", bufs=2)
                        nc.vector.tensor_mul(t1, dest[0:RD], cos_sb[:, js])
                        nc.vector.tensor_add(dest[0:RD], t1, t2)
                        yield

                # vtok = vhat^T (token-major v) via identity matmuls
                vtp = ps.tile([128, TC], F32, tag="QK", bufs=1, name=f"vtp{jn}")
                for kk in range(4):
                    nc.tensor.matmul(
                        vtp[:, kk * 128:(kk + 1) * 128],
                        vhat[jn][:, kk * 128:(kk + 1) * 128],
                        identb,
                        start=True, stop=True)
                    yield
                nc.scalar.copy(vtok[jn], vtp)
                yield

                # rvns = 1 / (sum_hd vhat^2 + eps), broadcast
                vsq = st.tile([128, TC], F32R, tag="vsq", bufs=1)
                nc.gpsimd.tensor_mul(vsq, vhat[jn], vhat[jn])
                vrow = ps.tile([1, TC], F32, tag="S", bufs=1, name=f"vrow{jn}")
                nc.tensor.matmul(vrow, ones_c, vsq, start=True, stop=True)
                yield
                vrow_sb = st.tile([1, TC], F32, tag="row", bufs=4, name="vrow_sb")
                nc.scalar.copy(vrow_sb, vrow)
                vb = st.tile([128, TC], F32, tag="bc", bufs=8, name="vb")
                nc.gpsimd.partition_broadcast(vb, vrow_sb)
                vb2 = st.tile([128, TC], F32, tag="bc", bufs=8, name="vb2")
                nc.vector.tensor_scalar_add(vb2, vb, EPS)
                nc.vector.reciprocal_approx_fast(rvnsb[jn], vb2)
                yield

            # ---------------- output projection for chunk jo --------------
            def outproj_gen(jo):
                r = jo % 2
                for tt in range(4):
                    for m in range(4):
                        po = ps.tile([128, TC], F32, tag="OP", bufs=1,
                                     name=f"po{jo}_{tt}_{m}")
                        for h in range(NH_L):
                            nc.tensor.matmul(
                                po,
                                aout[h][r][:, tt * 128:(tt + 1) * 128],
                                wo_sb[:, h * D + m * TC: h * D + (m + 1) * TC],
                                start=(h == 0), stop=(h == NH_L - 1),
                            )
                            if h % 2 == 1:
                                yield
                        osb = st.tile([128, TC], F32, tag="osb", bufs=4)
                        nc.vector.tensor_copy(osb, po)
                        nc.sync.dma_start(
                            out=outp[jo * TC + tt * 128: jo * TC + (tt + 1) * 128,
                                     m * TC:(m + 1) * TC],
                            in_=osb)
                        yield

            # ---------------- attention for chunk j ----------------
            def attention(j):
                r = j % 2
                nkt = 4 * (j + 1)
                pv = [ps.tile([128, TC], F32, tag="PV", bufs=4, name=f"pv{j}_{h}")
                      for h in range(NH_L)]
                for kt in range(nkt):
                    jk = kt // 4
                    ksl = slice((kt % 4) * 128, (kt % 4 + 1) * 128)
                    diag = (kt >= 4 * j)
                    for h in range(NH_L):
                        sc = ps.tile([128, TC], F32, tag="SC", bufs=1)
                        nc.tensor.matmul(sc, khat[jk][:, ksl], qhat[h][r],
                                         start=True, stop=(not diag))
                        if diag:
                            nc.tensor.matmul(sc, identb, maskM[kt - 4 * j],
                                             start=False, stop=True)
                        pt = st.tile([128, TC], BF16, tag="pT", bufs=4)
                        nc.scalar.activation(pt, sc, ACT.Exp, scale=S128)
                        if kt == 0:
                            nc.vector.tensor_copy(sumP[h], pt)
                        else:
                            nc.vector.tensor_add(sumP[h], sumP[h], pt)
                        emit_fill(2)
                        nc.tensor.matmul(pv[h], vtok[jk][:, ksl], pt,
                                         start=(kt == 0), stop=(kt == nkt - 1))

                # per-head epilogue: denominator + XSA correction
                for h in range(NH_L):
                    den = ps.tile([1, TC], F32, tag="S", bufs=1, name=f"den{j}_{h}")
                    nc.tensor.matmul(den, ones_c, sumP[h], start=True, stop=True)
                    den_sb = st.tile([1, TC], F32, tag="row", bufs=4, name="den_sb")
                    nc.scalar.copy(den_sb, den)
                    den_b = st.tile([128, TC], F32, tag="bc", bufs=8, name="den_b")
                    nc.gpsimd.partition_broadcast(den_b, den_sb)
                    inv_b = st.tile([128, TC], F32, tag="bc", bufs=8, name="inv_b")
                    nc.vector.reciprocal_approx_fast(inv_b, den_b)
                    pvs = st.tile([128, TC], F32, tag="pv", bufs=2, name="pvs")
                    nc.scalar.copy(pvs, pv[h])
                    tu = st.tile([128, TC], F32R, tag="tu", bufs=2, name="tu")
                    nc.gpsimd.tensor_mul(tu, pvs, vhat[j])
                    emit_fill(4)
                    dot = ps.tile([1, TC], F32, tag="S", bufs=1, name=f"dot{j}_{h}")
                    nc.tensor.matmul(dot, ones_c, tu, start=True, stop=True)
                    dot_sb = st.tile([1, TC], F32, tag="row", bufs=4, name="dot_sb")
                    nc.scalar.copy(dot_sb, dot)
                    dot_b = st.tile([128, TC], F32, tag="bc", bufs=8, name="dot_b")
                    nc.gpsimd.partition_broadcast(dot_b, dot_sb)
                    f_b = st.tile([128, TC], F32, tag="bc", bufs=8, name="f_b")
                    nc.vector.tensor_mul(f_b, dot_b, rvnsb[j])
                    m2 = st.tile([128, TC], F32, tag="m2", bufs=2, name="m2")
                    nc.vector.tensor_mul(m2, vhat[j], f_b)
                    nc.vector.tensor_sub(m2, pvs, m2)
                    nc.vector.tensor_mul(aout[h][r], m2, inv_b)
                    emit_fill(4)

            # ---------------- schedule ----------------
            # chunk 0 prep runs solid (nothing to interleave with)
            for _ in prep_gen(0):
                pass
            for j in range(NTC):
                if j + 1 < NTC:
                    fill_q.append(prep_gen(j + 1))
                if j >= 1:
                    fill_q.append(outproj_gen(j - 1))
                attention(j)
                drain_fill()
            for _ in outproj_gen(NTC - 1):
                pass

    nc.compile()
    return nc


def _host_inputs(x, cos, sin, w_norm, wq, wk, wv, wo):
    """Build the 8 per-core input maps (host-side layout prep only)."""
    bf = ml_dtypes.bfloat16
    wn = w_norm.astype(np.float32)
    cosT = cos.T.astype(np.float32)                                # [64, T]
    sinT = sin.T.astype(np.float32)
    sinS = np.concatenate([-sinT[:32], sinT[32:]], axis=0)         # [64, T]
    cs = np.ascontiguousarray(
        np.concatenate([cosT, sinS], axis=0), dtype=np.float32)    # [128, T]
    xb = x.astype(bf)
    xTs = [np.ascontiguousarray(xb[b].T) for b in range(B)]
    xRs = [np.ascontiguousarray(xb[b]) for b in range(B)]
    in_maps = []
    for c in range(8):
        b, g = divmod(c, 4)
        wq_s = wq[g * NH_L * HD:(g + 1) * NH_L * HD] * wn[None, :]
        wk_s = wk[g * HD:(g + 1) * HD] * wn[None, :]
        wv_s = wv[g * HD:(g + 1) * HD] * wn[None, :]
        wqkvT = np.ascontiguousarray(
            np.concatenate([wq_s, wk_s, wv_s], axis=0).T).astype(bf)  # [D, 768]
        # woL[p, h*D + d] = wo[d, g*512 + h*128 + p]
        woL = np.ascontiguousarray(
            wo[:, g * NH_L * HD:(g + 1) * NH_L * HD]                 # [D, 512]
            .T.reshape(NH_L, HD, D).transpose(1, 0, 2).reshape(HD, NH_L * D)
        ).astype(bf)                                                 # [128, 4*D]
        in_maps.append({
            "xT": xTs[b],
            "xR": xRs[b],
            "wqkvT": wqkvT,
            "woL": woL,
            "cs": cs,
        })
    return in_maps


def kernel(x, cos, sin, w_norm, wq, wk, wv, wo, rope_dims=64, use_xsa=1,
           **_unused):
    if "nc" not in _CACHE:
        _CACHE["nc"] = _build_nc()
    nc = _CACHE["nc"]
    in_maps = _host_inputs(
        np.asarray(x), np.asarray(cos), np.asarray(sin), np.asarray(w_norm),
        np.asarray(wq), np.asarray(wk), np.asarray(wv), np.asarray(wo),
    )
    res_obj = run_bass_kernel_spmd(nc, in_maps, list(range(8)))
    _CACHE["last"] = res_obj
    res = res_obj.results
    out = np.zeros((B, T, D), dtype=np.float32)
    for c in range(8):
        b = c // 4
        out[b] += np.asarray(res[c]["out"], dtype=np.float32)
    return out
